# revision 20
# baseline (speedup 1.0000x reference)
"""BitLinear (ternary group-quantized linear) Trainium2 Bass kernel.

Computes: w_q = groupwise_ternary_quantize(weight, group=128 along in_features)
          out = x @ w_q.T + bias
for x (4, 2048, 4096) f32, weight (16384, 4096) f32, bias (16384,) f32.

Sharding (tensor-parallel, per the row-sharding strategy):
  - weight rows (out_features) and bias sharded 8 ways: 2048 rows/core
  - x replicated to all 8 cores
  - each core computes its (8192, 2048) output slice; host concatenates.

Per-core kernel (SPMD, identical program, different input data):
  Phase C: cast x f32 -> bf16 with SWDGE casting DMAs (gpsimd, DRAM->DRAM,
           256KB pieces so they never head-of-line-block latency-critical
           loads), one scratch tile per 256-row block for dep granularity.
  Phase Q: quantize the 2048x4096 weight shard on-chip, f32 math so the
           ternary threshold decisions match the f32 reference:
           per-group |w| sums via ACT Abs+accum_out, scale = max(mean,eps),
           wq = (|w| > 0.5*scale) * scale * sign(w) on the vector engine,
           written bf16 to DRAM per 512-row strip, then ONE XBAR
           DMA-transpose per strip into an SBUF-resident K-major cache
           [128, 32, 512] x 4 (single writer per strip keeps the matmul
           wait chains trivial).
  Phase M: composable_matmul_tile_kernel, split (2 strips, 2 strips) so the
           first call starts as soon as half the cache is quantized:
           stationary = xT bf16 [128, 16, 256] tiles XBAR-DMA-transposed
           from the bf16 scratch (prefetched one token-batch ahead);
           moving = the SBUF cache; fp32 psum; bias (host-prebroadcast to
           [128, 2048]) added during psum->sbuf eviction; f32 out.

Measured on HW (NTFF profile, core 0): 2.40 ms/exec vs 1.75 ms pure-matmul
roofline (PE 76% busy).
"""

import os
from contextlib import ExitStack
from dataclasses import replace

import numpy as np

import concourse.bass as bass
import concourse.mybir as mybir
import concourse.tile as tile
from concourse import bacc
from concourse.bass import ds, ts
from concourse.bass_utils import run_bass_kernel_spmd
from concourse.kernels.tile_matmul import (
    ShapeInfo,
    composable_matmul_tile_kernel,
    dma_to_dram_mxn,
)

F32 = mybir.dt.float32
BF16 = mybir.dt.bfloat16
P = 128

N_CORES = 8
M_FULL = 8192          # 4*2048 tokens
K = 4096               # in_features
N_OUT_FULL = 16384     # out_features
N = N_OUT_FULL // N_CORES  # 2048 out rows per core
KG = K // P            # 32 contraction groups of 128 (also the quant groups)
MB = 256               # m batch (token block) size in phase M
N_STRIP = 512          # kxn cache strip width (= matmul N_TILE)
QK = 512               # k-chunk for the quant temps (SBUF pressure)


def build_kernel(
    tc: tile.TileContext,
    ctx: ExitStack,
    m_tokens: int,
    _skip_q: bool = False,
    _skip_c: bool = False,
    x_mode: str = "dma_cast",
    k_tile: int = 2048,
    kxm_bufs: int = 5,
    psum_n_bufs: int = 2,
    m_split: tuple = (2, 2),   # strips per composable call
):
    nc = tc.nc
    nb_m = m_tokens // MB
    n_rt = N // P            # 16 weight row-tiles
    n_strips = N // N_STRIP  # 4
    rts_per_strip = N_STRIP // P

    x_ap = nc.dram_tensor("x", [m_tokens, K], F32, kind="ExternalInput").ap()
    w_ap = nc.dram_tensor("w", [N, K], F32, kind="ExternalInput").ap()
    biasb_ap = nc.dram_tensor("biasb", [P, N], F32, kind="ExternalInput").ap()
    out_ap = nc.dram_tensor("out", [m_tokens, N], F32, kind="ExternalOutput").ap()

    const = ctx.enter_context(tc.tile_pool(name="const", bufs=1))
    cache_pool = ctx.enter_context(tc.tile_pool(name="kxncache", bufs=1))
    dram = ctx.enter_context(tc.tile_pool(name="dram", bufs=1, space="DRAM"))

    # K-major quantized-weight cache, SBUF resident: strip s holds out-rows
    # [512*s, 512*(s+1)) for all k: [p = k % 128, gk = k // 128, row]
    cache_strips = [
        cache_pool.tile([P, KG, N_STRIP], BF16, tag=f"kxnc{s}", name=f"kxnc{s}")
        for s in range(n_strips)
    ]
    # wq bf16 staging per strip; read back with one XBAR DMA-transpose per
    # strip into the SBUF cache (single writer -> single semaphore hop).
    wq_tiles = [
        dram.tile([N_STRIP, K], BF16, tag=f"wqd{s}", name=f"wqd{s}")
        for s in range(n_strips)
    ]
    # bf16 x scratch, one DRAM tile per 256-row block (dep granularity)
    xb_tiles = [
        dram.tile([MB, K], BF16, tag=f"xb{b}", name=f"xb{b}") for b in range(nb_m)
    ]

    biasb_sb = const.tile([P, N], F32, tag="biasb")
    nc.sync.dma_start(biasb_sb[:], biasb_ap)

    # ---------------- Phase C: cast x f32 -> bf16 (SWDGE casting DMA) ------
    cast_emitted = [False] * nb_m

    def emit_cast_block(b):
        if cast_emitted[b]:
            return
        cast_emitted[b] = True
        if _skip_c:
            return
        # Pieces small enough not to head-of-line-block latency-critical
        # loads on the shared DMA engines.
        for r in range(0, MB, 32):
            nc.gpsimd.dma_start(
                xb_tiles[b][ds(r, 32), :], x_ap[ds(b * MB + r, 32), :]
            )

    # ---------------- Phase Q: groupwise ternary quantization -------------
    q_pool = ctx.enter_context(tc.tile_pool(name="qp", bufs=3))
    qsmall = ctx.enter_context(tc.tile_pool(name="qsmall", bufs=2))

    def emit_q_strip(s):
        """Quantize out-rows [512s, 512(s+1)) and fill cache strip s."""
        if _skip_q:
            nc.any.memset(cache_strips[s][:], 0.0)
            return
        for rt in range(s * rts_per_strip, (s + 1) * rts_per_strip):
            col = (rt % rts_per_strip) * P
            for h in range(K // QK):
                gq = QK // P
                wf = q_pool.tile([P, gq, P], F32, tag="wf", name="wf")
                nc.sync.dma_start(wf[:], w_ap[ds(rt * P, P), ds(h * QK, QK)])
                # |w|, sign(w) and the per-group |w| sums all on the
                # (otherwise idle) scalar engine; accum_out yields each
                # group's sum as a side effect of the Abs pass.
                absw = q_pool.tile([P, gq, P], F32, tag="absw", name="absw")
                gsum = qsmall.tile([P, gq, 1], F32, tag="gsum", name="gsum")
                for g in range(gq):
                    nc.scalar.activation(
                        absw[:, g, :], wf[:, g, :],
                        mybir.ActivationFunctionType.Abs,
                        accum_out=gsum[:, g, :],
                    )
                sgw = q_pool.tile([P, gq, P], F32, tag="sgw", name="sgw")
                nc.scalar.activation(
                    sgw[:], wf[:], mybir.ActivationFunctionType.Sign
                )
                scale = qsmall.tile([P, gq, 1], F32, tag="scale", name="scale")
                nc.vector.tensor_scalar(
                    scale[:], gsum[:], 1.0 / P, 1e-8,
                    op0=mybir.AluOpType.mult, op1=mybir.AluOpType.max,
                )
                thr = qsmall.tile([P, gq, 1], F32, tag="thr", name="thr")
                nc.vector.tensor_scalar(
                    thr[:], scale[:], 0.5, None, op0=mybir.AluOpType.mult
                )
                # wq = (|w| > 0.5*scale) * scale * sign(w), in place on absw
                _, thr_b = bass.broadcast_tensor_aps(absw[:], thr[:])
                nc.vector.tensor_tensor(
                    absw[:], absw[:], thr_b, op=mybir.AluOpType.is_gt
                )
                _, scale_b = bass.broadcast_tensor_aps(absw[:], scale[:])
                nc.vector.tensor_tensor(
                    absw[:], absw[:], scale_b, op=mybir.AluOpType.mult
                )
                wqb = q_pool.tile([P, gq, P], BF16, tag="wqb", name="wqb")
                nc.vector.tensor_tensor(
                    wqb[:], absw[:], sgw[:], op=mybir.AluOpType.mult
                )
                nc.sync.dma_start(
                    wq_tiles[s][ds(col, P), ds(h * QK, QK)], wqb[:]
                )
        src = wq_tiles[s][:].rearrange("f (po pi) -> f po pi", pi=P)
        nc.sync.dma_start_transpose(cache_strips[s][:], src)

    # ---------------- Phase M machinery -----------------------------------
    kxm_pool = ctx.enter_context(tc.tile_pool(name="kxm", bufs=kxm_bufs))
    ksub = k_tile // P
    k_tiles = K // k_tile
    CAST_AHEAD = 4
    LOAD_AHEAD = int(os.environ.get("KXM_LOAD_AHEAD", "1"))

    def emit_kxm_load(cache, b, kt):
        t = kxm_pool.tile([P, ksub, MB], BF16, tag="xkxm", name="xkxm")
        src = xb_tiles[b][:].rearrange("f (po pi) -> f po pi", pi=P)
        nc.sync.dma_start_transpose(t[:], src[:, ts(kt, ksub), :])
        cache[(b, kt)] = t

    def run_m_call(strip_base, strips_in_call):
        width = strips_in_call * N_STRIP
        kcache = {}

        def kxm_producer(nc_, md):
            b, kt = md.m_batch_idx, md.k_tile_idx
            if (b, kt) not in kcache:
                emit_kxm_load(kcache, b, kt)
            t = kcache.pop((b, kt))
            if kt == 0:
                nb = b + LOAD_AHEAD
                if nb < nb_m:
                    for nkt in range(k_tiles):
                        if (nb, nkt) not in kcache:
                            emit_kxm_load(kcache, nb, nkt)
                nxt = b + CAST_AHEAD
                if nxt < nb_m:
                    emit_cast_block(nxt)
            return t

        def kxn_producer(nc_, md):
            assert md.n_tile == N_STRIP and md.n_batch_idx == 0
            s = strip_base + md.n_tile_idx
            return cache_strips[s][:, ts(md.k_tile_idx, md.k_subtiles), :]

        consumers = [
            dma_to_dram_mxn(out_ap[ds(b * MB, MB), ds(strip_base * N_STRIP, width)])
            for b in range(nb_m)
        ]

        def mxn_consumer(nc_, sbuf_tile, md):
            consumers[md.m_batch_idx](nc_, sbuf_tile, replace(md, m_batch_idx=0))

        def bias_reducer(nc_, psum, sbuf, md):
            off = (strip_base + md.n_tile_idx) * N_STRIP + md.n_subtile_idx * md.n_subtile
            nc_.vector.tensor_tensor(
                out=sbuf[:, 0, :],
                in0=psum,
                in1=biasb_sb[:, ds(off, md.n_subtile)],
                op=mybir.AluOpType.add,
            )

        composable_matmul_tile_kernel(
            tc=tc,
            kxm_shape=ShapeInfo(pdims=((P, KG),), fdims=(MB,) * nb_m),
            kxn_shape=ShapeInfo(pdims=((P, KG),), fdims=(width,)),
            output_type=F32,
            kxm_producer=kxm_producer,
            kxn_producer=kxn_producer,
            mxn_consumer=mxn_consumer,
            mxn_subtile_reducer=bias_reducer,
            MATMUL_FREE_DIM=512,
            MAX_TILE_SIZE=512,
            MAX_K_TILE_SIZE=k_tile,
            cache_tiles=True,
            temps_n_bufs=2,
            psum_n_bufs=psum_n_bufs,
        )

    # ---------------- Emission schedule -----------------------------------
    # Quantize the first strip block, start matmuling it while the remaining
    # strips quantize, then matmul the rest.
    assert sum(m_split) == n_strips
    base = 0
    for ci, cnt in enumerate(m_split):
        for st in range(base, base + cnt):
            emit_q_strip(st)
        if ci == 0:
            for b in range(min(CAST_AHEAD, nb_m)):
                emit_cast_block(b)
        run_m_call(base, cnt)
        base += cnt


def build_program(m_tokens: int = M_FULL, **kw):
    nc = bacc.Bacc(
        "TRN2",
        target_bir_lowering=False,
        debug=False,
        enable_asserts=False,
        num_devices=N_CORES,
    )
    with tile.TileContext(nc) as tc, ExitStack() as ctx:
        build_kernel(tc, ctx, m_tokens, **kw)
    nc.compile()
    return nc


_program_cache = {}


def _get_program(m_tokens: int):
    if m_tokens not in _program_cache:
        _program_cache[m_tokens] = build_program(m_tokens)
    return _program_cache[m_tokens]


def make_in_maps(x: np.ndarray, weight: np.ndarray, bias: np.ndarray):
    """Shard the full inputs for the 8 cores: replicate x, split w/bias rows."""
    xf = np.ascontiguousarray(x.reshape(-1, K).astype(np.float32, copy=False))
    in_maps = []
    for c in range(N_CORES):
        wsh = np.ascontiguousarray(weight[c * N:(c + 1) * N])
        bsh = bias[c * N:(c + 1) * N]
        biasb = np.ascontiguousarray(
            np.broadcast_to(bsh[None, :], (P, N)).astype(np.float32, copy=False)
        )
        in_maps.append({"x": xf, "w": wsh, "biasb": biasb})
    return in_maps


def kernel(x: np.ndarray, weight: np.ndarray, bias: np.ndarray):
    nc = _get_program(x.shape[0] * x.shape[1])
    in_maps = make_in_maps(x, weight, bias)
    res = run_bass_kernel_spmd(nc, in_maps, core_ids=list(range(N_CORES)))
    out = np.concatenate([res.results[c]["out"] for c in range(N_CORES)], axis=1)
    kernel.last_results = res
    return out.reshape(x.shape[0], x.shape[1], N_OUT_FULL).astype(np.float32)
